# revision 11
# baseline (speedup 1.0000x reference)
"""Additive (Bahdanau) attention on 8 Trainium2 NeuronCores.

Reference computation (per batch b):
    kp = key[:, b, :] @ Wk            (S, H)
    qp = query[:, b, :] @ Wq + bk+bq  (T, H)
    scores[t, s] = sum_h v[h] * tanh(qp[t, h] + kp[s, h])
    out[b] = softmax(scores, axis=s)  (T, S)

Sharding: batch B=8 across the 8 cores, one batch element per core.
No collectives needed.

Device layout: H on SBUF partitions (2 halves of 128).  The broadcast
add + tanh is fused into a single ScalarE ACTIVATE per (t, half) using
the per-partition bias operand: E = tanh(kp[h, s] + qp_t[h]), reading
kp straight from PSUM.  The reduction over h runs on the TensorEngine
with a shifted-v stationary operand (v in column t%32 routes score_t to
PSUM partition t); softmax is fused via reduce_max(negate) +
Exp(accum_out).  PE operands are fp16 (measured end-to-end rel err
1.5e-3 on the reference data): fp32 matmuls lower to two HW passes,
fp16 to one.
"""

import numpy as np

S, T, B = 1024, 64, 8
D, H = 512, 256
N_CORES = 8
PART = 128
ND = D // PART       # 4 contraction chunks
NH = H // PART       # 2 h halves
NSC = S // 512       # matmul N<=512 chunks (PSUM bank limit)

_CACHE: dict = {}


def _build_nc():
    import concourse.bass as bass
    import concourse.tile as tile
    from concourse import bacc, mybir

    f32 = mybir.dt.float32
    f16 = mybir.dt.float16
    AF = mybir.ActivationFunctionType
    AX = mybir.AxisListType
    ALU = mybir.AluOpType

    nc = bacc.Bacc(
        "TRN2",
        target_bir_lowering=False,
        debug=False,
        num_devices=N_CORES,
    )

    keyT = nc.dram_tensor("keyT", [D, S], f16, kind="ExternalInput").ap()
    queryT = nc.dram_tensor("queryT", [D, T], f16, kind="ExternalInput").ap()
    wk = nc.dram_tensor("wk", [D, H], f16, kind="ExternalInput").ap()
    wq = nc.dram_tensor("wq", [D, H], f16, kind="ExternalInput").ap()
    bsum = nc.dram_tensor("bsum", [H], f32, kind="ExternalInput").ap()
    # vshift[h, p, j, m] = v[h*128+p] if j == m else 0
    vshift = nc.dram_tensor("vshift", [NH, PART, 32, 32], f16, kind="ExternalInput").ap()
    out = nc.dram_tensor("out", [T, S], f32, kind="ExternalOutput").ap()

    with tile.TileContext(nc) as tc:
        with (
            tc.tile_pool(name="const", bufs=1) as cpool,
            tc.tile_pool(name="epool", bufs=4) as epool,
            tc.tile_pool(name="spool", bufs=1) as spool,
            tc.tile_pool(name="kp_ps", bufs=1, space="PSUM") as kp_pool,
            tc.tile_pool(name="ps_small", bufs=2, space="PSUM") as ps_small,
        ):
            # ---- load inputs; qp path (scalar HWDGE queue) goes first ----
            queryT_sb = cpool.tile([PART, ND, T], f16)
            nc.scalar.dma_start(queryT_sb[:], queryT.rearrange("(n p) t -> p n t", p=PART))
            wq_sb = cpool.tile([PART, ND, H], f16)
            nc.scalar.dma_start(wq_sb[:], wq.rearrange("(n p) h -> p n h", p=PART))
            wk_sb = cpool.tile([PART, ND, H], f16)
            nc.sync.dma_start(wk_sb[:], wk.rearrange("(n p) h -> p n h", p=PART))
            keyT_sb = cpool.tile([PART, ND, S], f16)
            keyT_r = keyT.rearrange("(n p) s -> p n s", p=PART)
            for n in range(ND):
                eng = nc.sync if n % 2 == 0 else nc.scalar
                eng.dma_start(keyT_sb[:, n, :], keyT_r[:, n, :])
            bsum_sb = cpool.tile([PART, NH], f32)
            nc.gpsimd.dma_start(bsum_sb[:], bsum.rearrange("(a p) -> p a", p=PART))
            vshift_sb = cpool.tile([PART, NH, 32, 32], f16)
            nc.gpsimd.dma_start(vshift_sb[:], vshift.rearrange("h p j m -> p h j m"))

            # ---- qp^T [h, t] first (small, unblocks the main loop) ----
            qp_sb = []
            for h in range(NH):
                qp_ps = ps_small.tile([PART, T], f32, tag="qp", name=f"qp_ps{h}")
                for n in range(ND):
                    nc.tensor.matmul(
                        qp_ps[:],
                        wq_sb[:, n, h * PART:(h + 1) * PART],
                        queryT_sb[:, n, :],
                        start=(n == 0),
                        stop=(n == ND - 1),
                    )
                q = cpool.tile([PART, T], f32, tag=f"qp_sb{h}", name=f"qp_sb{h}")
                # qp + (bk+bq), fused into the PSUM->SBUF copy
                nc.scalar.add(q[:], qp_ps[:], bsum_sb[:, h:h + 1])
                qp_sb.append(q)

            # ---- kp^T [h, s], PSUM-resident for the whole main loop ----
            kp_ps = [
                kp_pool.tile([PART, S], f32, tag=f"kp{h}", name=f"kp{h}")
                for h in range(NH)
            ]
            for h in range(NH):
                for c in range(NSC):
                    for n in range(ND):
                        nc.tensor.matmul(
                            kp_ps[h][:, c * 512:(c + 1) * 512],
                            wk_sb[:, n, h * PART:(h + 1) * PART],
                            keyT_sb[:, n, c * 512:(c + 1) * 512],
                            start=(n == 0),
                            stop=(n == ND - 1),
                        )

            # ---- main loop: E = tanh(kp + qp_t); scores[t] += v . E ----
            # h outer so kp_ps[1] is computed while the h=0 pass runs.
            scores_ps = ps_small.tile([T, S], f32, tag="qp")
            for h in range(NH):
                for t in range(T):
                    g, j = divmod(t, 32)
                    e = epool.tile([PART, S], f16, tag="e", name=f"e_{t}_{h}")
                    nc.scalar.activation(
                        e[:], kp_ps[h][:], AF.Tanh, bias=qp_sb[h][:, t:t + 1]
                    )
                    for c in range(NSC):
                        nc.tensor.matmul(
                            scores_ps[32 * g:32 * (g + 1), c * 512:(c + 1) * 512],
                            vshift_sb[:, h, j, :],
                            e[:, c * 512:(c + 1) * 512],
                            start=(j == 0 and h == 0),
                            stop=(j == 31 and h == NH - 1),
                            skip_group_check=True,
                        )

            # ---- softmax over s (free axis), t on partitions ----
            negmax = spool.tile([T, 1], f32)
            nc.vector.tensor_reduce(
                negmax[:], scores_ps[:], axis=AX.X, op=ALU.max, negate=True,
            )
            p_sb = spool.tile([T, S], f32)
            ssum = spool.tile([T, 1], f32)
            nc.scalar.activation(
                p_sb[:], scores_ps[:], AF.Exp, bias=negmax[:], accum_out=ssum[:]
            )
            rinv = spool.tile([T, 1], f32)
            nc.vector.reciprocal(rinv[:], ssum[:])
            out_sb = spool.tile([T, S], f32)
            nc.vector.tensor_scalar_mul(out_sb[:], p_sb[:], rinv[:])
            nc.sync.dma_start(out[:], out_sb[:])

    nc.compile()
    return nc


def _get_nc():
    if "nc" not in _CACHE:
        _CACHE["nc"] = _build_nc()
    return _CACHE["nc"]


def _in_maps(key, query, Wk, bk, Wq, bq, v):
    key = np.asarray(key, dtype=np.float32)
    query = np.asarray(query, dtype=np.float32)
    keyT = np.ascontiguousarray(
        key.transpose(1, 2, 0).astype(np.float16))      # (B, D, S)
    queryT = np.ascontiguousarray(
        query.transpose(1, 2, 0).astype(np.float16))    # (B, D, T)
    wk = np.ascontiguousarray(np.asarray(Wk, dtype=np.float32).astype(np.float16))
    wq = np.ascontiguousarray(np.asarray(Wq, dtype=np.float32).astype(np.float16))
    bsum = np.asarray(bk, dtype=np.float32) + np.asarray(bq, dtype=np.float32)
    vv = np.asarray(v, dtype=np.float32)
    vshift = np.zeros((NH, PART, 32, 32), dtype=np.float16)
    for h in range(NH):
        for j in range(32):
            vshift[h, :, j, j] = vv[h * PART:(h + 1) * PART]
    return [
        {
            "keyT": keyT[b], "queryT": queryT[b],
            "wk": wk, "wq": wq, "bsum": bsum, "vshift": vshift,
        }
        for b in range(N_CORES)
    ]


def kernel(key, query, Wk, bk, Wq, bq, v):
    from concourse.bass_utils import run_bass_kernel_spmd

    nc = _get_nc()
    in_maps = _in_maps(key, query, Wk, bk, Wq, bq, v)
    res = run_bass_kernel_spmd(nc, in_maps, core_ids=list(range(N_CORES)))
    return np.stack([res.results[b]["out"] for b in range(N_CORES)])


def _ensure_ntff_hook():
    """Provide antenv.axon_hooks (absent in this image) so that
    run_bass_kernel_spmd(trace=True) can drive NTFF profiling via the
    libaxon_pjrt.so C ABI directly."""
    import sys
    import types
    import ctypes
    import contextlib

    try:
        from antenv.axon_hooks import get_axon_ntff_profile_hook  # noqa: F401
        return
    except ImportError:
        pass

    import antenv

    holder = {}
    mod = types.ModuleType("antenv.axon_hooks")
    mod.set_axon_ntff_profile_hook = lambda h: holder.__setitem__("h", h)
    mod.get_axon_ntff_profile_hook = lambda: holder.get("h")
    sys.modules["antenv.axon_hooks"] = mod
    antenv.axon_hooks = mod

    so_path = "/opt/axon/libaxon_pjrt.so"
    lib = ctypes.CDLL(so_path)
    if not hasattr(lib, "axon_start_nrt_profile"):
        return
    lib.axon_start_nrt_profile.argtypes = [
        ctypes.POINTER(ctypes.c_int64),
        ctypes.c_size_t,
    ]
    lib.axon_start_nrt_profile.restype = ctypes.c_int64
    lib.axon_stop_nrt_profile.argtypes = [ctypes.c_char_p]
    lib.axon_stop_nrt_profile.restype = ctypes.c_int64

    @contextlib.contextmanager
    def _hook(output_dir, device_ids):
        import jax

        jax.devices()
        if device_ids:
            ids = (ctypes.c_int64 * len(device_ids))(*device_ids)
            rc = lib.axon_start_nrt_profile(ids, len(device_ids))
        else:
            rc = lib.axon_start_nrt_profile(None, 0)
        if rc != 0:
            raise RuntimeError(f"axon_start_nrt_profile rc={rc}")
        try:
            yield
        finally:
            n = lib.axon_stop_nrt_profile(str(output_dir).encode())
            print(f"ntff profile: {n} file(s) written to {output_dir}")

    mod.set_axon_ntff_profile_hook(_hook)


def kernel_traced(key, query, Wk, bk, Wq, bq, v):
    """Same as kernel() but captures the neuron profile; returns
    (output, exec_time_ns, trace_path)."""
    from concourse.bass_utils import run_bass_kernel_spmd

    _ensure_ntff_hook()
    nc = _get_nc()
    in_maps = _in_maps(key, query, Wk, bk, Wq, bq, v)
    res = run_bass_kernel_spmd(
        nc, in_maps, core_ids=list(range(N_CORES)), trace=True
    )
    outp = np.stack([res.results[b]["out"] for b in range(N_CORES)])
    trace_path = None
    if res.instructions_and_trace is not None:
        trace_path = res.instructions_and_trace[1]
    return outp, res.exec_time_ns, trace_path


# revision 12
# speedup vs baseline: 1.0024x; 1.0024x over previous
"""Additive (Bahdanau) attention on 8 Trainium2 NeuronCores.

Reference computation (per batch b):
    kp = key[:, b, :] @ Wk            (S, H)
    qp = query[:, b, :] @ Wq + bk+bq  (T, H)
    scores[t, s] = sum_h v[h] * tanh(qp[t, h] + kp[s, h])
    out[b] = softmax(scores, axis=s)  (T, S)

Sharding: batch B=8 across the 8 cores, one batch element per core.
No collectives needed.

Device layout: H on SBUF partitions (2 halves of 128).  The broadcast
add + tanh is fused into a single ScalarE ACTIVATE per (t, half) using
the per-partition bias operand: E = tanh(kp[h, s] + qp_t[h]), reading
kp straight from PSUM.  The reduction over h runs on the TensorEngine
with a shifted-v stationary operand (v in column t%32 routes score_t to
PSUM partition t); softmax is fused via reduce_max(negate) +
Exp(accum_out).  PE operands are fp16 (measured end-to-end rel err
1.5e-3 on the reference data): fp32 matmuls lower to two HW passes,
fp16 to one.
"""

import numpy as np

S, T, B = 1024, 64, 8
D, H = 512, 256
N_CORES = 8
PART = 128
ND = D // PART       # 4 contraction chunks
NH = H // PART       # 2 h halves
NSC = S // 512       # matmul N<=512 chunks (PSUM bank limit)

_CACHE: dict = {}


def _build_nc():
    import concourse.bass as bass
    import concourse.tile as tile
    from concourse import bacc, mybir

    f32 = mybir.dt.float32
    f16 = mybir.dt.float16
    AF = mybir.ActivationFunctionType
    AX = mybir.AxisListType
    ALU = mybir.AluOpType

    nc = bacc.Bacc(
        "TRN2",
        target_bir_lowering=False,
        debug=False,
        num_devices=N_CORES,
    )

    keyT = nc.dram_tensor("keyT", [D, S], f16, kind="ExternalInput").ap()
    queryT = nc.dram_tensor("queryT", [D, T], f16, kind="ExternalInput").ap()
    wk = nc.dram_tensor("wk", [D, H], f16, kind="ExternalInput").ap()
    wq = nc.dram_tensor("wq", [D, H], f16, kind="ExternalInput").ap()
    bsum = nc.dram_tensor("bsum", [H], f32, kind="ExternalInput").ap()
    # vshift[h, p, j, m] = v[h*128+p] if j == m else 0
    bf16 = mybir.dt.bfloat16
    vshift = nc.dram_tensor("vshift", [NH, PART, 32, 32], bf16, kind="ExternalInput").ap()
    out = nc.dram_tensor("out", [T, S], f32, kind="ExternalOutput").ap()

    with tile.TileContext(nc) as tc:
        with (
            tc.tile_pool(name="const", bufs=1) as cpool,
            tc.tile_pool(name="epool", bufs=4) as epool,
            tc.tile_pool(name="spool", bufs=1) as spool,
            tc.tile_pool(name="kp_ps", bufs=1, space="PSUM") as kp_pool,
            tc.tile_pool(name="ps_small", bufs=2, space="PSUM") as ps_small,
        ):
            # ---- load inputs; qp path (scalar HWDGE queue) goes first ----
            queryT_sb = cpool.tile([PART, ND, T], f16)
            nc.scalar.dma_start(queryT_sb[:], queryT.rearrange("(n p) t -> p n t", p=PART))
            wq_sb = cpool.tile([PART, ND, H], f16)
            nc.scalar.dma_start(wq_sb[:], wq.rearrange("(n p) h -> p n h", p=PART))
            wk_sb = cpool.tile([PART, ND, H], f16)
            nc.sync.dma_start(wk_sb[:], wk.rearrange("(n p) h -> p n h", p=PART))
            keyT_sb = cpool.tile([PART, ND, S], f16)
            keyT_r = keyT.rearrange("(n p) s -> p n s", p=PART)
            for n in range(ND):
                eng = nc.sync if n % 2 == 0 else nc.scalar
                eng.dma_start(keyT_sb[:, n, :], keyT_r[:, n, :])
            bsum_sb = cpool.tile([PART, NH], f32)
            nc.gpsimd.dma_start(bsum_sb[:], bsum.rearrange("(a p) -> p a", p=PART))
            vshift_sb = cpool.tile([PART, NH, 32, 32], bf16)
            nc.gpsimd.dma_start(vshift_sb[:], vshift.rearrange("h p j m -> p h j m"))

            # ---- qp^T [h, t] first (small, unblocks the main loop) ----
            qp_sb = []
            for h in range(NH):
                qp_ps = ps_small.tile([PART, T], f32, tag="qp", name=f"qp_ps{h}")
                for n in range(ND):
                    nc.tensor.matmul(
                        qp_ps[:],
                        wq_sb[:, n, h * PART:(h + 1) * PART],
                        queryT_sb[:, n, :],
                        start=(n == 0),
                        stop=(n == ND - 1),
                    )
                q = cpool.tile([PART, T], f32, tag=f"qp_sb{h}", name=f"qp_sb{h}")
                # qp + (bk+bq), fused into the PSUM->SBUF copy
                nc.scalar.add(q[:], qp_ps[:], bsum_sb[:, h:h + 1])
                qp_sb.append(q)

            # ---- kp^T [h, s], PSUM-resident for the whole main loop ----
            kp_ps = [
                kp_pool.tile([PART, S], f32, tag=f"kp{h}", name=f"kp{h}")
                for h in range(NH)
            ]
            for h in range(NH):
                for c in range(NSC):
                    for n in range(ND):
                        nc.tensor.matmul(
                            kp_ps[h][:, c * 512:(c + 1) * 512],
                            wk_sb[:, n, h * PART:(h + 1) * PART],
                            keyT_sb[:, n, c * 512:(c + 1) * 512],
                            start=(n == 0),
                            stop=(n == ND - 1),
                        )

            # ---- main loop: E = tanh(kp + qp_t); scores[t] += v . E ----
            # h outer so kp_ps[1] is computed while the h=0 pass runs.
            scores_ps = ps_small.tile([T, S], f32, tag="qp")
            for h in range(NH):
                for t in range(T):
                    g, j = divmod(t, 32)
                    e = epool.tile([PART, S], bf16, tag="e", name=f"e_{t}_{h}")
                    nc.scalar.activation(
                        e[:], kp_ps[h][:], AF.Tanh, bias=qp_sb[h][:, t:t + 1]
                    )
                    for c in range(NSC):
                        nc.tensor.matmul(
                            scores_ps[32 * g:32 * (g + 1), c * 512:(c + 1) * 512],
                            vshift_sb[:, h, j, :],
                            e[:, c * 512:(c + 1) * 512],
                            start=(j == 0 and h == 0),
                            stop=(j == 31 and h == NH - 1),
                            skip_group_check=True,
                        )

            # ---- softmax over s (free axis), t on partitions ----
            negmax = spool.tile([T, 1], f32)
            nc.vector.tensor_reduce(
                negmax[:], scores_ps[:], axis=AX.X, op=ALU.max, negate=True,
            )
            p_sb = spool.tile([T, S], f32)
            ssum = spool.tile([T, 1], f32)
            nc.scalar.activation(
                p_sb[:], scores_ps[:], AF.Exp, bias=negmax[:], accum_out=ssum[:]
            )
            rinv = spool.tile([T, 1], f32)
            nc.vector.reciprocal(rinv[:], ssum[:])
            out_sb = spool.tile([T, S], f32)
            nc.vector.tensor_scalar_mul(out_sb[:], p_sb[:], rinv[:])
            nc.sync.dma_start(out[:], out_sb[:])

    nc.compile()
    return nc


def _get_nc():
    if "nc" not in _CACHE:
        _CACHE["nc"] = _build_nc()
    return _CACHE["nc"]


def _in_maps(key, query, Wk, bk, Wq, bq, v):
    key = np.asarray(key, dtype=np.float32)
    query = np.asarray(query, dtype=np.float32)
    keyT = np.ascontiguousarray(
        key.transpose(1, 2, 0).astype(np.float16))      # (B, D, S)
    queryT = np.ascontiguousarray(
        query.transpose(1, 2, 0).astype(np.float16))    # (B, D, T)
    wk = np.ascontiguousarray(np.asarray(Wk, dtype=np.float32).astype(np.float16))
    wq = np.ascontiguousarray(np.asarray(Wq, dtype=np.float32).astype(np.float16))
    bsum = np.asarray(bk, dtype=np.float32) + np.asarray(bq, dtype=np.float32)
    vv = np.asarray(v, dtype=np.float32)
    import ml_dtypes
    vshift = np.zeros((NH, PART, 32, 32), dtype=ml_dtypes.bfloat16)
    for h in range(NH):
        for j in range(32):
            vshift[h, :, j, j] = vv[h * PART:(h + 1) * PART]
    return [
        {
            "keyT": keyT[b], "queryT": queryT[b],
            "wk": wk, "wq": wq, "bsum": bsum, "vshift": vshift,
        }
        for b in range(N_CORES)
    ]


def kernel(key, query, Wk, bk, Wq, bq, v):
    from concourse.bass_utils import run_bass_kernel_spmd

    nc = _get_nc()
    in_maps = _in_maps(key, query, Wk, bk, Wq, bq, v)
    res = run_bass_kernel_spmd(nc, in_maps, core_ids=list(range(N_CORES)))
    return np.stack([res.results[b]["out"] for b in range(N_CORES)])


def _ensure_ntff_hook():
    """Provide antenv.axon_hooks (absent in this image) so that
    run_bass_kernel_spmd(trace=True) can drive NTFF profiling via the
    libaxon_pjrt.so C ABI directly."""
    import sys
    import types
    import ctypes
    import contextlib

    try:
        from antenv.axon_hooks import get_axon_ntff_profile_hook  # noqa: F401
        return
    except ImportError:
        pass

    import antenv

    holder = {}
    mod = types.ModuleType("antenv.axon_hooks")
    mod.set_axon_ntff_profile_hook = lambda h: holder.__setitem__("h", h)
    mod.get_axon_ntff_profile_hook = lambda: holder.get("h")
    sys.modules["antenv.axon_hooks"] = mod
    antenv.axon_hooks = mod

    so_path = "/opt/axon/libaxon_pjrt.so"
    lib = ctypes.CDLL(so_path)
    if not hasattr(lib, "axon_start_nrt_profile"):
        return
    lib.axon_start_nrt_profile.argtypes = [
        ctypes.POINTER(ctypes.c_int64),
        ctypes.c_size_t,
    ]
    lib.axon_start_nrt_profile.restype = ctypes.c_int64
    lib.axon_stop_nrt_profile.argtypes = [ctypes.c_char_p]
    lib.axon_stop_nrt_profile.restype = ctypes.c_int64

    @contextlib.contextmanager
    def _hook(output_dir, device_ids):
        import jax

        jax.devices()
        if device_ids:
            ids = (ctypes.c_int64 * len(device_ids))(*device_ids)
            rc = lib.axon_start_nrt_profile(ids, len(device_ids))
        else:
            rc = lib.axon_start_nrt_profile(None, 0)
        if rc != 0:
            raise RuntimeError(f"axon_start_nrt_profile rc={rc}")
        try:
            yield
        finally:
            n = lib.axon_stop_nrt_profile(str(output_dir).encode())
            print(f"ntff profile: {n} file(s) written to {output_dir}")

    mod.set_axon_ntff_profile_hook(_hook)


def kernel_traced(key, query, Wk, bk, Wq, bq, v):
    """Same as kernel() but captures the neuron profile; returns
    (output, exec_time_ns, trace_path)."""
    from concourse.bass_utils import run_bass_kernel_spmd

    _ensure_ntff_hook()
    nc = _get_nc()
    in_maps = _in_maps(key, query, Wk, bk, Wq, bq, v)
    res = run_bass_kernel_spmd(
        nc, in_maps, core_ids=list(range(N_CORES)), trace=True
    )
    outp = np.stack([res.results[b]["out"] for b in range(N_CORES)])
    trace_path = None
    if res.instructions_and_trace is not None:
        trace_path = res.instructions_and_trace[1]
    return outp, res.exec_time_ns, trace_path


# revision 13
# speedup vs baseline: 1.1168x; 1.1141x over previous
"""Additive (Bahdanau) attention on 8 Trainium2 NeuronCores.

Reference computation (per batch b):
    kp = key[:, b, :] @ Wk            (S, H)
    qp = query[:, b, :] @ Wq + bk+bq  (T, H)
    scores[t, s] = sum_h v[h] * tanh(qp[t, h] + kp[s, h])
    out[b] = softmax(scores, axis=s)  (T, S)

Sharding: batch B=8 across the 8 cores, one batch element per core.
No collectives needed.

Device layout: H on SBUF partitions (2 halves of 128).  The broadcast
add + tanh is fused into a single ScalarE ACTIVATE per (t, half) using
the per-partition bias operand: E = tanh(kp[h, s] + qp_t[h]), reading
kp straight from PSUM.  The reduction over h runs on the TensorEngine
with a shifted-v stationary operand (v in column t%32 routes score_t to
PSUM partition t); softmax is fused via reduce_max(negate) +
Exp(accum_out).  PE operands are fp16 (measured end-to-end rel err
1.5e-3 on the reference data): fp32 matmuls lower to two HW passes,
fp16 to one.
"""

import numpy as np

S, T, B = 1024, 64, 8
D, H = 512, 256
N_CORES = 8
PART = 128
ND = D // PART       # 4 contraction chunks
NH = H // PART       # 2 h halves
NSC = S // 512       # matmul N<=512 chunks (PSUM bank limit)

_CACHE: dict = {}


def _build_nc():
    import concourse.bass as bass
    import concourse.tile as tile
    from concourse import bacc, mybir

    f32 = mybir.dt.float32
    f16 = mybir.dt.float16
    AF = mybir.ActivationFunctionType
    AX = mybir.AxisListType
    ALU = mybir.AluOpType

    nc = bacc.Bacc(
        "TRN2",
        target_bir_lowering=False,
        debug=False,
        num_devices=N_CORES,
    )

    keyT = nc.dram_tensor("keyT", [D, S], f16, kind="ExternalInput").ap()
    queryT = nc.dram_tensor("queryT", [D, T], f16, kind="ExternalInput").ap()
    wk = nc.dram_tensor("wk", [D, H], f16, kind="ExternalInput").ap()
    wq = nc.dram_tensor("wq", [D, H], f16, kind="ExternalInput").ap()
    bsum = nc.dram_tensor("bsum", [H], f32, kind="ExternalInput").ap()
    # vshift[h, p, j, m] = v[h*128+p] if j == m else 0
    bf16 = mybir.dt.bfloat16
    vshift = nc.dram_tensor("vshift", [NH, PART, 32, 32], bf16, kind="ExternalInput").ap()
    out = nc.dram_tensor("out", [T, S], f32, kind="ExternalOutput").ap()

    with tile.TileContext(nc) as tc:
        with (
            tc.tile_pool(name="const", bufs=1) as cpool,
            tc.tile_pool(name="epool", bufs=4) as epool,
            tc.tile_pool(name="spool", bufs=1) as spool,
            tc.tile_pool(name="kp_ps", bufs=1, space="PSUM") as kp_pool,
            tc.tile_pool(name="ps_small", bufs=2, space="PSUM") as ps_small,
        ):
            # ---- load inputs; qp path (scalar HWDGE queue) goes first ----
            queryT_sb = cpool.tile([PART, ND, T], f16)
            nc.scalar.dma_start(queryT_sb[:], queryT.rearrange("(n p) t -> p n t", p=PART))
            wq_sb = cpool.tile([PART, ND, H], f16)
            nc.scalar.dma_start(wq_sb[:], wq.rearrange("(n p) h -> p n h", p=PART))
            wk_sb = cpool.tile([PART, ND, H], f16)
            nc.sync.dma_start(wk_sb[:], wk.rearrange("(n p) h -> p n h", p=PART))
            keyT_sb = cpool.tile([PART, ND, S], f16)
            keyT_r = keyT.rearrange("(n p) s -> p n s", p=PART)
            for n in range(ND):
                eng = nc.sync if n % 2 == 0 else nc.scalar
                eng.dma_start(keyT_sb[:, n, :], keyT_r[:, n, :])
            bsum_sb = cpool.tile([PART, NH], f32)
            nc.gpsimd.dma_start(bsum_sb[:], bsum.rearrange("(a p) -> p a", p=PART))
            vshift_sb = cpool.tile([PART, NH, 32, 32], bf16)
            nc.gpsimd.dma_start(vshift_sb[:], vshift.rearrange("h p j m -> p h j m"))

            # ---- qp^T [h, t] first (small, unblocks the main loop) ----
            qp_sb = []
            for h in range(NH):
                qp_ps = ps_small.tile([PART, T], f32, tag="qp", name=f"qp_ps{h}")
                for n in range(ND):
                    nc.tensor.matmul(
                        qp_ps[:],
                        wq_sb[:, n, h * PART:(h + 1) * PART],
                        queryT_sb[:, n, :],
                        start=(n == 0),
                        stop=(n == ND - 1),
                    )
                q = cpool.tile([PART, T], f32, tag=f"qp_sb{h}", name=f"qp_sb{h}")
                # qp + (bk+bq), fused into the PSUM->SBUF copy
                nc.scalar.add(q[:], qp_ps[:], bsum_sb[:, h:h + 1])
                qp_sb.append(q)

            # ---- kp^T [h, s], PSUM-resident for the whole main loop ----
            kp_ps = [
                kp_pool.tile([PART, S], f32, tag=f"kp{h}", name=f"kp{h}")
                for h in range(NH)
            ]
            for h in range(NH):
                for c in range(NSC):
                    for n in range(ND):
                        nc.tensor.matmul(
                            kp_ps[h][:, c * 512:(c + 1) * 512],
                            wk_sb[:, n, h * PART:(h + 1) * PART],
                            keyT_sb[:, n, c * 512:(c + 1) * 512],
                            start=(n == 0),
                            stop=(n == ND - 1),
                        )

            # ---- main loop: E = tanh(kp + qp_t); scores[t] += v . E ----
            # h outer so kp_ps[1] is computed while the h=0 pass runs.
            scores_ps = ps_small.tile([T, S], f32, tag="qp")
            for t in range(T):
                for h in range(NH):
                    g, j = divmod(t, 32)
                    e = epool.tile([PART, S], bf16, tag="e", name=f"e_{t}_{h}")
                    nc.scalar.activation(
                        e[:], kp_ps[h][:], AF.Tanh, bias=qp_sb[h][:, t:t + 1]
                    )
                    for c in range(NSC):
                        nc.tensor.matmul(
                            scores_ps[32 * g:32 * (g + 1), c * 512:(c + 1) * 512],
                            vshift_sb[:, h, j, :],
                            e[:, c * 512:(c + 1) * 512],
                            start=(j == 0 and h == 0),
                            stop=(j == 31 and h == NH - 1),
                            skip_group_check=True,
                        )

            # ---- softmax over s (free axis), t on partitions ----
            negmax = spool.tile([T, 1], f32)
            nc.vector.tensor_reduce(
                negmax[:], scores_ps[:], axis=AX.X, op=ALU.max, negate=True,
            )
            p_sb = spool.tile([T, S], f32)
            ssum = spool.tile([T, 1], f32)
            nc.scalar.activation(
                p_sb[:], scores_ps[:], AF.Exp, bias=negmax[:], accum_out=ssum[:]
            )
            rinv = spool.tile([T, 1], f32)
            nc.vector.reciprocal(rinv[:], ssum[:])
            out_sb = spool.tile([T, S], f32)
            nc.vector.tensor_scalar_mul(out_sb[:], p_sb[:], rinv[:])
            nc.sync.dma_start(out[:], out_sb[:])

    nc.compile()
    return nc


def _get_nc():
    if "nc" not in _CACHE:
        _CACHE["nc"] = _build_nc()
    return _CACHE["nc"]


def _in_maps(key, query, Wk, bk, Wq, bq, v):
    key = np.asarray(key, dtype=np.float32)
    query = np.asarray(query, dtype=np.float32)
    keyT = np.ascontiguousarray(
        key.transpose(1, 2, 0).astype(np.float16))      # (B, D, S)
    queryT = np.ascontiguousarray(
        query.transpose(1, 2, 0).astype(np.float16))    # (B, D, T)
    wk = np.ascontiguousarray(np.asarray(Wk, dtype=np.float32).astype(np.float16))
    wq = np.ascontiguousarray(np.asarray(Wq, dtype=np.float32).astype(np.float16))
    bsum = np.asarray(bk, dtype=np.float32) + np.asarray(bq, dtype=np.float32)
    vv = np.asarray(v, dtype=np.float32)
    import ml_dtypes
    vshift = np.zeros((NH, PART, 32, 32), dtype=ml_dtypes.bfloat16)
    for h in range(NH):
        for j in range(32):
            vshift[h, :, j, j] = vv[h * PART:(h + 1) * PART]
    return [
        {
            "keyT": keyT[b], "queryT": queryT[b],
            "wk": wk, "wq": wq, "bsum": bsum, "vshift": vshift,
        }
        for b in range(N_CORES)
    ]


def kernel(key, query, Wk, bk, Wq, bq, v):
    from concourse.bass_utils import run_bass_kernel_spmd

    nc = _get_nc()
    in_maps = _in_maps(key, query, Wk, bk, Wq, bq, v)
    res = run_bass_kernel_spmd(nc, in_maps, core_ids=list(range(N_CORES)))
    return np.stack([res.results[b]["out"] for b in range(N_CORES)])


def _ensure_ntff_hook():
    """Provide antenv.axon_hooks (absent in this image) so that
    run_bass_kernel_spmd(trace=True) can drive NTFF profiling via the
    libaxon_pjrt.so C ABI directly."""
    import sys
    import types
    import ctypes
    import contextlib

    try:
        from antenv.axon_hooks import get_axon_ntff_profile_hook  # noqa: F401
        return
    except ImportError:
        pass

    import antenv

    holder = {}
    mod = types.ModuleType("antenv.axon_hooks")
    mod.set_axon_ntff_profile_hook = lambda h: holder.__setitem__("h", h)
    mod.get_axon_ntff_profile_hook = lambda: holder.get("h")
    sys.modules["antenv.axon_hooks"] = mod
    antenv.axon_hooks = mod

    so_path = "/opt/axon/libaxon_pjrt.so"
    lib = ctypes.CDLL(so_path)
    if not hasattr(lib, "axon_start_nrt_profile"):
        return
    lib.axon_start_nrt_profile.argtypes = [
        ctypes.POINTER(ctypes.c_int64),
        ctypes.c_size_t,
    ]
    lib.axon_start_nrt_profile.restype = ctypes.c_int64
    lib.axon_stop_nrt_profile.argtypes = [ctypes.c_char_p]
    lib.axon_stop_nrt_profile.restype = ctypes.c_int64

    @contextlib.contextmanager
    def _hook(output_dir, device_ids):
        import jax

        jax.devices()
        if device_ids:
            ids = (ctypes.c_int64 * len(device_ids))(*device_ids)
            rc = lib.axon_start_nrt_profile(ids, len(device_ids))
        else:
            rc = lib.axon_start_nrt_profile(None, 0)
        if rc != 0:
            raise RuntimeError(f"axon_start_nrt_profile rc={rc}")
        try:
            yield
        finally:
            n = lib.axon_stop_nrt_profile(str(output_dir).encode())
            print(f"ntff profile: {n} file(s) written to {output_dir}")

    mod.set_axon_ntff_profile_hook(_hook)


def kernel_traced(key, query, Wk, bk, Wq, bq, v):
    """Same as kernel() but captures the neuron profile; returns
    (output, exec_time_ns, trace_path)."""
    from concourse.bass_utils import run_bass_kernel_spmd

    _ensure_ntff_hook()
    nc = _get_nc()
    in_maps = _in_maps(key, query, Wk, bk, Wq, bq, v)
    res = run_bass_kernel_spmd(
        nc, in_maps, core_ids=list(range(N_CORES)), trace=True
    )
    outp = np.stack([res.results[b]["out"] for b in range(N_CORES)])
    trace_path = None
    if res.instructions_and_trace is not None:
        trace_path = res.instructions_and_trace[1]
    return outp, res.exec_time_ns, trace_path


# revision 16
# speedup vs baseline: 1.1282x; 1.0102x over previous
"""Additive (Bahdanau) attention on 8 Trainium2 NeuronCores.

Reference computation (per batch b):
    kp = key[:, b, :] @ Wk            (S, H)
    qp = query[:, b, :] @ Wq + bk+bq  (T, H)
    scores[t, s] = sum_h v[h] * tanh(qp[t, h] + kp[s, h])
    out[b] = softmax(scores, axis=s)  (T, S)

Sharding: batch B=8 across the 8 cores, one batch element per core.
No collectives needed.

Device layout: H on SBUF partitions (2 halves of 128).  The broadcast
add + tanh is fused into a single ScalarE ACTIVATE per (t, half) using
the per-partition bias operand: E = tanh(kp[h, s] + qp_t[h]), reading
kp straight from PSUM.  The reduction over h runs on the TensorEngine
with a shifted-v stationary operand (v in column t%32 routes score_t to
PSUM partition t); softmax is fused via reduce_max(negate) +
Exp(accum_out).  PE operands are fp16 (measured end-to-end rel err
1.5e-3 on the reference data): fp32 matmuls lower to two HW passes,
fp16 to one.
"""

import numpy as np

S, T, B = 1024, 64, 8
D, H = 512, 256
N_CORES = 8
PART = 128
ND = D // PART       # 4 contraction chunks
NH = H // PART       # 2 h halves
NSC = S // 512       # matmul N<=512 chunks (PSUM bank limit)

_CACHE: dict = {}


def _build_nc():
    import concourse.bass as bass
    import concourse.tile as tile
    from concourse import bacc, mybir

    f32 = mybir.dt.float32
    f16 = mybir.dt.float16
    AF = mybir.ActivationFunctionType
    AX = mybir.AxisListType
    ALU = mybir.AluOpType

    nc = bacc.Bacc(
        "TRN2",
        target_bir_lowering=False,
        debug=False,
        num_devices=N_CORES,
    )

    keyT = nc.dram_tensor("keyT", [D, S], f16, kind="ExternalInput").ap()
    queryT = nc.dram_tensor("queryT", [D, T], f16, kind="ExternalInput").ap()
    wk = nc.dram_tensor("wk", [D, H], f16, kind="ExternalInput").ap()
    wq = nc.dram_tensor("wq", [D, H], f16, kind="ExternalInput").ap()
    bsum = nc.dram_tensor("bsum", [H], f32, kind="ExternalInput").ap()
    # vshift[h, p, j, m] = v[h*128+p] if j == m else 0
    bf16 = mybir.dt.bfloat16
    vshift = nc.dram_tensor("vshift", [NH, PART, 32, 32], bf16, kind="ExternalInput").ap()
    out = nc.dram_tensor("out", [T, S], f32, kind="ExternalOutput").ap()

    with tile.TileContext(nc) as tc:
        with (
            tc.tile_pool(name="const", bufs=1) as cpool,
            tc.tile_pool(name="epool", bufs=4) as epool,
            tc.tile_pool(name="spool", bufs=1) as spool,
            tc.tile_pool(name="kp_ps", bufs=1, space="PSUM") as kp_pool,
            tc.tile_pool(name="ps_small", bufs=2, space="PSUM") as ps_small,
        ):
            # ---- PE warmup: dep-free matmuls on uninitialized scratch keep
            # the PE busy during the input DMAs so HAM un-throttles to
            # 2.4 GHz before the real matmuls start.
            warm_w = cpool.tile([PART, 512], f16)
            nc.gpsimd.memset(warm_w[:], 0.0)
            warm_ps = kp_pool.tile([PART, 512], f32, tag="kp0", name="warm_ps")
            for _ in range(28):
                nc.tensor.matmul(
                    warm_ps[:], warm_w[:, 0:PART], warm_w[:],
                    start=True, stop=True, skip_group_check=True,
                )

            # ---- load inputs; qp path (scalar HWDGE queue) goes first ----
            queryT_sb = cpool.tile([PART, ND, T], f16)
            nc.scalar.dma_start(queryT_sb[:], queryT.rearrange("(n p) t -> p n t", p=PART))
            wq_sb = cpool.tile([PART, ND, H], f16)
            nc.scalar.dma_start(wq_sb[:], wq.rearrange("(n p) h -> p n h", p=PART))
            wk_sb = cpool.tile([PART, ND, H], f16)
            nc.sync.dma_start(wk_sb[:], wk.rearrange("(n p) h -> p n h", p=PART))
            keyT_sb = cpool.tile([PART, ND, S], f16)
            keyT_r = keyT.rearrange("(n p) s -> p n s", p=PART)
            for n in range(ND):
                nc.sync.dma_start(keyT_sb[:, n, :], keyT_r[:, n, :])
            bsum_sb = cpool.tile([PART, NH], f32)
            nc.gpsimd.dma_start(bsum_sb[:], bsum.rearrange("(a p) -> p a", p=PART))
            vshift_sb = cpool.tile([PART, NH, 32, 32], bf16)
            nc.gpsimd.dma_start(vshift_sb[:], vshift.rearrange("h p j m -> p h j m"))

            # ---- qp^T [h, t] first (small, unblocks the main loop) ----
            qp_sb = []
            for h in range(NH):
                qp_ps = ps_small.tile([PART, T], f32, tag="qp", name=f"qp_ps{h}")
                for n in range(ND):
                    nc.tensor.matmul(
                        qp_ps[:],
                        wq_sb[:, n, h * PART:(h + 1) * PART],
                        queryT_sb[:, n, :],
                        start=(n == 0),
                        stop=(n == ND - 1),
                    )
                q = cpool.tile([PART, T], f32, tag=f"qp_sb{h}", name=f"qp_sb{h}")
                # qp + (bk+bq), fused into the PSUM->SBUF copy
                nc.scalar.add(q[:], qp_ps[:], bsum_sb[:, h:h + 1])
                qp_sb.append(q)

            # ---- kp^T [h, s], PSUM-resident for the whole main loop ----
            kp_ps = [
                kp_pool.tile([PART, S], f32, tag=f"kp{h}", name=f"kp{h}")
                for h in range(NH)
            ]
            for h in range(NH):
                for c in range(NSC):
                    for n in range(ND):
                        nc.tensor.matmul(
                            kp_ps[h][:, c * 512:(c + 1) * 512],
                            wk_sb[:, n, h * PART:(h + 1) * PART],
                            keyT_sb[:, n, c * 512:(c + 1) * 512],
                            start=(n == 0),
                            stop=(n == ND - 1),
                        )

            # ---- main loop: E = tanh(kp + qp_t); scores[t] += v . E ----
            # h outer so kp_ps[1] is computed while the h=0 pass runs.
            scores_ps = [
                ps_small.tile([32, S], f32, tag="qp", name=f"scores{g}")
                for g in range(2)
            ]
            for t in range(T):
                g, j = divmod(t, 32)
                for h in range(NH):
                    e = epool.tile([PART, S], bf16, tag="e", name=f"e_{t}_{h}")
                    nc.scalar.activation(
                        e[:], kp_ps[h][:], AF.Tanh, bias=qp_sb[h][:, t:t + 1]
                    )
                    for c in range(NSC):
                        nc.tensor.matmul(
                            scores_ps[g][:, c * 512:(c + 1) * 512],
                            vshift_sb[:, h, j, :],
                            e[:, c * 512:(c + 1) * 512],
                            start=(j == 0 and h == 0),
                            stop=(j == 31 and h == NH - 1),
                        )

            # ---- softmax over s (free axis), per 32-query group; group 0
            # is complete halfway through the main loop and overlaps it ----
            for g in range(2):
                negmax = spool.tile([32, 1], f32, tag=f"nm{g}", name=f"negmax{g}")
                nc.vector.tensor_reduce(
                    negmax[:], scores_ps[g][:], axis=AX.X, op=ALU.max, negate=True,
                )
                p_sb = spool.tile([32, S], f32, tag=f"p{g}", name=f"p_sb{g}")
                ssum = spool.tile([32, 1], f32, tag=f"ss{g}", name=f"ssum{g}")
                nc.scalar.activation(
                    p_sb[:], scores_ps[g][:], AF.Exp, bias=negmax[:],
                    accum_out=ssum[:],
                )
                rinv = spool.tile([32, 1], f32, tag=f"ri{g}", name=f"rinv{g}")
                nc.vector.reciprocal(rinv[:], ssum[:])
                out_sb = spool.tile([32, S], f32, tag=f"ob{g}", name=f"out_sb{g}")
                nc.vector.tensor_scalar_mul(out_sb[:], p_sb[:], rinv[:])
                nc.sync.dma_start(out[32 * g:32 * (g + 1), :], out_sb[:])

    nc.compile()
    return nc


def _get_nc():
    if "nc" not in _CACHE:
        _CACHE["nc"] = _build_nc()
    return _CACHE["nc"]


def _in_maps(key, query, Wk, bk, Wq, bq, v):
    key = np.asarray(key, dtype=np.float32)
    query = np.asarray(query, dtype=np.float32)
    keyT = np.ascontiguousarray(
        key.transpose(1, 2, 0).astype(np.float16))      # (B, D, S)
    queryT = np.ascontiguousarray(
        query.transpose(1, 2, 0).astype(np.float16))    # (B, D, T)
    wk = np.ascontiguousarray(np.asarray(Wk, dtype=np.float32).astype(np.float16))
    wq = np.ascontiguousarray(np.asarray(Wq, dtype=np.float32).astype(np.float16))
    bsum = np.asarray(bk, dtype=np.float32) + np.asarray(bq, dtype=np.float32)
    vv = np.asarray(v, dtype=np.float32)
    import ml_dtypes
    vshift = np.zeros((NH, PART, 32, 32), dtype=ml_dtypes.bfloat16)
    for h in range(NH):
        for j in range(32):
            vshift[h, :, j, j] = vv[h * PART:(h + 1) * PART]
    return [
        {
            "keyT": keyT[b], "queryT": queryT[b],
            "wk": wk, "wq": wq, "bsum": bsum, "vshift": vshift,
        }
        for b in range(N_CORES)
    ]


def kernel(key, query, Wk, bk, Wq, bq, v):
    from concourse.bass_utils import run_bass_kernel_spmd

    nc = _get_nc()
    in_maps = _in_maps(key, query, Wk, bk, Wq, bq, v)
    res = run_bass_kernel_spmd(nc, in_maps, core_ids=list(range(N_CORES)))
    return np.stack([res.results[b]["out"] for b in range(N_CORES)])


def _ensure_ntff_hook():
    """Provide antenv.axon_hooks (absent in this image) so that
    run_bass_kernel_spmd(trace=True) can drive NTFF profiling via the
    libaxon_pjrt.so C ABI directly."""
    import sys
    import types
    import ctypes
    import contextlib

    try:
        from antenv.axon_hooks import get_axon_ntff_profile_hook  # noqa: F401
        return
    except ImportError:
        pass

    import antenv

    holder = {}
    mod = types.ModuleType("antenv.axon_hooks")
    mod.set_axon_ntff_profile_hook = lambda h: holder.__setitem__("h", h)
    mod.get_axon_ntff_profile_hook = lambda: holder.get("h")
    sys.modules["antenv.axon_hooks"] = mod
    antenv.axon_hooks = mod

    so_path = "/opt/axon/libaxon_pjrt.so"
    lib = ctypes.CDLL(so_path)
    if not hasattr(lib, "axon_start_nrt_profile"):
        return
    lib.axon_start_nrt_profile.argtypes = [
        ctypes.POINTER(ctypes.c_int64),
        ctypes.c_size_t,
    ]
    lib.axon_start_nrt_profile.restype = ctypes.c_int64
    lib.axon_stop_nrt_profile.argtypes = [ctypes.c_char_p]
    lib.axon_stop_nrt_profile.restype = ctypes.c_int64

    @contextlib.contextmanager
    def _hook(output_dir, device_ids):
        import jax

        jax.devices()
        if device_ids:
            ids = (ctypes.c_int64 * len(device_ids))(*device_ids)
            rc = lib.axon_start_nrt_profile(ids, len(device_ids))
        else:
            rc = lib.axon_start_nrt_profile(None, 0)
        if rc != 0:
            raise RuntimeError(f"axon_start_nrt_profile rc={rc}")
        try:
            yield
        finally:
            n = lib.axon_stop_nrt_profile(str(output_dir).encode())
            print(f"ntff profile: {n} file(s) written to {output_dir}")

    mod.set_axon_ntff_profile_hook(_hook)


def kernel_traced(key, query, Wk, bk, Wq, bq, v):
    """Same as kernel() but captures the neuron profile; returns
    (output, exec_time_ns, trace_path)."""
    from concourse.bass_utils import run_bass_kernel_spmd

    _ensure_ntff_hook()
    nc = _get_nc()
    in_maps = _in_maps(key, query, Wk, bk, Wq, bq, v)
    res = run_bass_kernel_spmd(
        nc, in_maps, core_ids=list(range(N_CORES)), trace=True
    )
    outp = np.stack([res.results[b]["out"] for b in range(N_CORES)])
    trace_path = None
    if res.instructions_and_trace is not None:
        trace_path = res.instructions_and_trace[1]
    return outp, res.exec_time_ns, trace_path


# revision 17
# speedup vs baseline: 1.1306x; 1.0021x over previous
"""Additive (Bahdanau) attention on 8 Trainium2 NeuronCores.

Reference computation (per batch b):
    kp = key[:, b, :] @ Wk            (S, H)
    qp = query[:, b, :] @ Wq + bk+bq  (T, H)
    scores[t, s] = sum_h v[h] * tanh(qp[t, h] + kp[s, h])
    out[b] = softmax(scores, axis=s)  (T, S)

Sharding: batch B=8 across the 8 cores, one batch element per core.
No collectives needed.

Device layout: H on SBUF partitions (2 halves of 128).  The broadcast
add + tanh is fused into a single ScalarE ACTIVATE per (t, half) using
the per-partition bias operand: E = tanh(kp[h, s] + qp_t[h]), reading
kp straight from PSUM.  The reduction over h runs on the TensorEngine
with a shifted-v stationary operand (v in column t%32 routes score_t to
PSUM partition t); softmax is fused via reduce_max(negate) +
Exp(accum_out).  PE operands are fp16 (measured end-to-end rel err
1.5e-3 on the reference data): fp32 matmuls lower to two HW passes,
fp16 to one.
"""

import numpy as np

S, T, B = 1024, 64, 8
D, H = 512, 256
N_CORES = 8
PART = 128
ND = D // PART       # 4 contraction chunks
NH = H // PART       # 2 h halves
NSC = S // 512       # matmul N<=512 chunks (PSUM bank limit)

_CACHE: dict = {}


def _build_nc():
    import concourse.bass as bass
    import concourse.tile as tile
    from concourse import bacc, mybir

    f32 = mybir.dt.float32
    f16 = mybir.dt.float16
    AF = mybir.ActivationFunctionType
    AX = mybir.AxisListType
    ALU = mybir.AluOpType

    nc = bacc.Bacc(
        "TRN2",
        target_bir_lowering=False,
        debug=False,
        num_devices=N_CORES,
    )

    # All inputs arrive pre-arranged host-side into [partition, free...]
    # SBUF layout so every input DMA is a maximal-linearity copy.
    keyT = nc.dram_tensor("keyT", [PART, ND, S], f16, kind="ExternalInput").ap()
    queryT = nc.dram_tensor("queryT", [PART, ND, T], f16, kind="ExternalInput").ap()
    wk = nc.dram_tensor("wk", [PART, ND, H], f16, kind="ExternalInput").ap()
    wq = nc.dram_tensor("wq", [PART, ND, H], f16, kind="ExternalInput").ap()
    bsum = nc.dram_tensor("bsum", [PART, NH], f32, kind="ExternalInput").ap()
    # vshift[p, h, j, m] = v[h*128+p] if j == m else 0
    bf16 = mybir.dt.bfloat16
    vshift = nc.dram_tensor("vshift", [PART, NH, 32, 32], bf16, kind="ExternalInput").ap()
    out = nc.dram_tensor("out", [T, S], f32, kind="ExternalOutput").ap()

    with tile.TileContext(nc) as tc:
        with (
            tc.tile_pool(name="const", bufs=1) as cpool,
            tc.tile_pool(name="epool", bufs=4) as epool,
            tc.tile_pool(name="spool", bufs=1) as spool,
            tc.tile_pool(name="kp_ps", bufs=1, space="PSUM") as kp_pool,
            tc.tile_pool(name="ps_small", bufs=2, space="PSUM") as ps_small,
        ):
            # ---- PE warmup: dep-free matmuls on uninitialized scratch keep
            # the PE busy during the input DMAs so HAM un-throttles to
            # 2.4 GHz before the real matmuls start.
            warm_w = cpool.tile([PART, 512], f16)
            nc.gpsimd.memset(warm_w[:], 0.0)
            warm_ps = kp_pool.tile([PART, 512], f32, tag="kp0", name="warm_ps")
            for _ in range(28):
                nc.tensor.matmul(
                    warm_ps[:], warm_w[:, 0:PART], warm_w[:],
                    start=True, stop=True, skip_group_check=True,
                )

            # ---- load inputs; qp path (scalar HWDGE queue) goes first ----
            queryT_sb = cpool.tile([PART, ND, T], f16)
            nc.scalar.dma_start(queryT_sb[:], queryT[:])
            wq_sb = cpool.tile([PART, ND, H], f16)
            nc.scalar.dma_start(wq_sb[:], wq[:])
            wk_sb = cpool.tile([PART, ND, H], f16)
            nc.sync.dma_start(wk_sb[:], wk[:])
            keyT_sb = cpool.tile([PART, ND, S], f16)
            for n in range(ND):
                eng = nc.sync if n < 2 else nc.scalar
                eng.dma_start(keyT_sb[:, n, :], keyT[:, n, :])
            bsum_sb = cpool.tile([PART, NH], f32)
            nc.gpsimd.dma_start(bsum_sb[:], bsum[:])
            vshift_sb = cpool.tile([PART, NH, 32, 32], bf16)
            nc.gpsimd.dma_start(vshift_sb[:], vshift[:])

            # ---- qp^T [h, t] first (small, unblocks the main loop) ----
            qp_sb = []
            for h in range(NH):
                qp_ps = ps_small.tile([PART, T], f32, tag="qp", name=f"qp_ps{h}")
                for n in range(ND):
                    nc.tensor.matmul(
                        qp_ps[:],
                        wq_sb[:, n, h * PART:(h + 1) * PART],
                        queryT_sb[:, n, :],
                        start=(n == 0),
                        stop=(n == ND - 1),
                    )
                q = cpool.tile([PART, T], f32, tag=f"qp_sb{h}", name=f"qp_sb{h}")
                # qp + (bk+bq), fused into the PSUM->SBUF copy
                nc.scalar.add(q[:], qp_ps[:], bsum_sb[:, h:h + 1])
                qp_sb.append(q)

            # ---- kp^T [h, s], PSUM-resident for the whole main loop ----
            kp_ps = [
                kp_pool.tile([PART, S], f32, tag=f"kp{h}", name=f"kp{h}")
                for h in range(NH)
            ]
            for h in range(NH):
                for c in range(NSC):
                    for n in range(ND):
                        nc.tensor.matmul(
                            kp_ps[h][:, c * 512:(c + 1) * 512],
                            wk_sb[:, n, h * PART:(h + 1) * PART],
                            keyT_sb[:, n, c * 512:(c + 1) * 512],
                            start=(n == 0),
                            stop=(n == ND - 1),
                        )

            # ---- main loop: E = tanh(kp + qp_t); scores[t] += v . E ----
            # h outer so kp_ps[1] is computed while the h=0 pass runs.
            scores_ps = [
                ps_small.tile([32, S], f32, tag="qp", name=f"scores{g}")
                for g in range(2)
            ]
            for t in range(T):
                g, j = divmod(t, 32)
                for h in range(NH):
                    e = epool.tile([PART, S], bf16, tag="e", name=f"e_{t}_{h}")
                    nc.scalar.activation(
                        e[:], kp_ps[h][:], AF.Tanh, bias=qp_sb[h][:, t:t + 1]
                    )
                    for c in range(NSC):
                        nc.tensor.matmul(
                            scores_ps[g][:, c * 512:(c + 1) * 512],
                            vshift_sb[:, h, j, :],
                            e[:, c * 512:(c + 1) * 512],
                            start=(j == 0 and h == 0),
                            stop=(j == 31 and h == NH - 1),
                        )

            # ---- softmax over s (free axis), per 32-query group; group 0
            # is complete halfway through the main loop and overlaps it ----
            for g in range(2):
                negmax = spool.tile([32, 1], f32, tag=f"nm{g}", name=f"negmax{g}")
                nc.vector.tensor_reduce(
                    negmax[:], scores_ps[g][:], axis=AX.X, op=ALU.max, negate=True,
                )
                p_sb = spool.tile([32, S], f32, tag=f"p{g}", name=f"p_sb{g}")
                ssum = spool.tile([32, 1], f32, tag=f"ss{g}", name=f"ssum{g}")
                nc.scalar.activation(
                    p_sb[:], scores_ps[g][:], AF.Exp, bias=negmax[:],
                    accum_out=ssum[:],
                )
                rinv = spool.tile([32, 1], f32, tag=f"ri{g}", name=f"rinv{g}")
                nc.vector.reciprocal(rinv[:], ssum[:])
                out_sb = spool.tile([32, S], f32, tag=f"ob{g}", name=f"out_sb{g}")
                nc.vector.tensor_scalar_mul(out_sb[:], p_sb[:], rinv[:])
                nc.sync.dma_start(out[32 * g:32 * (g + 1), :], out_sb[:])

    nc.compile()
    return nc


def _get_nc():
    if "nc" not in _CACHE:
        _CACHE["nc"] = _build_nc()
    return _CACHE["nc"]


def _part_layout(mat):
    """(X, F) with X = n*128+p  ->  contiguous (128, n, F) fp16."""
    x, f = mat.shape
    n = x // PART
    return np.ascontiguousarray(
        mat.reshape(n, PART, f).transpose(1, 0, 2).astype(np.float16))


def _in_maps(key, query, Wk, bk, Wq, bq, v):
    key = np.asarray(key, dtype=np.float32)
    query = np.asarray(query, dtype=np.float32)
    keyT = [_part_layout(key[:, b, :].T) for b in range(B)]     # (128, ND, S)
    queryT = [_part_layout(query[:, b, :].T) for b in range(B)]  # (128, ND, T)
    wk = _part_layout(np.asarray(Wk, dtype=np.float32))
    wq = _part_layout(np.asarray(Wq, dtype=np.float32))
    bsum0 = np.asarray(bk, dtype=np.float32) + np.asarray(bq, dtype=np.float32)
    bsum = np.ascontiguousarray(bsum0.reshape(NH, PART).T)       # (128, NH)
    vv = np.asarray(v, dtype=np.float32)
    import ml_dtypes
    vshift = np.zeros((PART, NH, 32, 32), dtype=ml_dtypes.bfloat16)
    for h in range(NH):
        for j in range(32):
            vshift[:, h, j, j] = vv[h * PART:(h + 1) * PART]
    return [
        {
            "keyT": keyT[b], "queryT": queryT[b],
            "wk": wk, "wq": wq, "bsum": bsum, "vshift": vshift,
        }
        for b in range(N_CORES)
    ]


def kernel(key, query, Wk, bk, Wq, bq, v):
    from concourse.bass_utils import run_bass_kernel_spmd

    nc = _get_nc()
    in_maps = _in_maps(key, query, Wk, bk, Wq, bq, v)
    res = run_bass_kernel_spmd(nc, in_maps, core_ids=list(range(N_CORES)))
    return np.stack([res.results[b]["out"] for b in range(N_CORES)])


def _ensure_ntff_hook():
    """Provide antenv.axon_hooks (absent in this image) so that
    run_bass_kernel_spmd(trace=True) can drive NTFF profiling via the
    libaxon_pjrt.so C ABI directly."""
    import sys
    import types
    import ctypes
    import contextlib

    try:
        from antenv.axon_hooks import get_axon_ntff_profile_hook  # noqa: F401
        return
    except ImportError:
        pass

    import antenv

    holder = {}
    mod = types.ModuleType("antenv.axon_hooks")
    mod.set_axon_ntff_profile_hook = lambda h: holder.__setitem__("h", h)
    mod.get_axon_ntff_profile_hook = lambda: holder.get("h")
    sys.modules["antenv.axon_hooks"] = mod
    antenv.axon_hooks = mod

    so_path = "/opt/axon/libaxon_pjrt.so"
    lib = ctypes.CDLL(so_path)
    if not hasattr(lib, "axon_start_nrt_profile"):
        return
    lib.axon_start_nrt_profile.argtypes = [
        ctypes.POINTER(ctypes.c_int64),
        ctypes.c_size_t,
    ]
    lib.axon_start_nrt_profile.restype = ctypes.c_int64
    lib.axon_stop_nrt_profile.argtypes = [ctypes.c_char_p]
    lib.axon_stop_nrt_profile.restype = ctypes.c_int64

    @contextlib.contextmanager
    def _hook(output_dir, device_ids):
        import jax

        jax.devices()
        if device_ids:
            ids = (ctypes.c_int64 * len(device_ids))(*device_ids)
            rc = lib.axon_start_nrt_profile(ids, len(device_ids))
        else:
            rc = lib.axon_start_nrt_profile(None, 0)
        if rc != 0:
            raise RuntimeError(f"axon_start_nrt_profile rc={rc}")
        try:
            yield
        finally:
            n = lib.axon_stop_nrt_profile(str(output_dir).encode())
            print(f"ntff profile: {n} file(s) written to {output_dir}")

    mod.set_axon_ntff_profile_hook(_hook)


def kernel_traced(key, query, Wk, bk, Wq, bq, v):
    """Same as kernel() but captures the neuron profile; returns
    (output, exec_time_ns, trace_path)."""
    from concourse.bass_utils import run_bass_kernel_spmd

    _ensure_ntff_hook()
    nc = _get_nc()
    in_maps = _in_maps(key, query, Wk, bk, Wq, bq, v)
    res = run_bass_kernel_spmd(
        nc, in_maps, core_ids=list(range(N_CORES)), trace=True
    )
    outp = np.stack([res.results[b]["out"] for b in range(N_CORES)])
    trace_path = None
    if res.instructions_and_trace is not None:
        trace_path = res.instructions_and_trace[1]
    return outp, res.exec_time_ns, trace_path


# revision 18
# speedup vs baseline: 1.1798x; 1.0435x over previous
"""Additive (Bahdanau) attention on 8 Trainium2 NeuronCores.

Reference computation (per batch b):
    kp = key[:, b, :] @ Wk            (S, H)
    qp = query[:, b, :] @ Wq + bk+bq  (T, H)
    scores[t, s] = sum_h v[h] * tanh(qp[t, h] + kp[s, h])
    out[b] = softmax(scores, axis=s)  (T, S)

Sharding: batch B=8 across the 8 cores, one batch element per core.
No collectives needed.

Device layout: H on SBUF partitions (2 halves of 128).  The broadcast
add + tanh is fused into a single ScalarE ACTIVATE per (t, half) using
the per-partition bias operand: E = tanh(kp[h, s] + qp_t[h]), reading
kp straight from PSUM.  The reduction over h runs on the TensorEngine
with a shifted-v stationary operand (v in column t%32 routes score_t to
PSUM partition t); softmax is fused via reduce_max(negate) +
Exp(accum_out).  PE operands are fp16 (measured end-to-end rel err
1.5e-3 on the reference data): fp32 matmuls lower to two HW passes,
fp16 to one.
"""

import numpy as np

S, T, B = 1024, 64, 8
D, H = 512, 256
N_CORES = 8
PART = 128
ND = D // PART       # 4 contraction chunks
NH = H // PART       # 2 h halves
NSC = S // 512       # matmul N<=512 chunks (PSUM bank limit)

_CACHE: dict = {}


def _build_nc():
    import concourse.bass as bass
    import concourse.tile as tile
    from concourse import bacc, mybir

    f32 = mybir.dt.float32
    f16 = mybir.dt.float16
    AF = mybir.ActivationFunctionType
    AX = mybir.AxisListType
    ALU = mybir.AluOpType

    nc = bacc.Bacc(
        "TRN2",
        target_bir_lowering=False,
        debug=False,
        num_devices=N_CORES,
    )

    # All inputs arrive pre-arranged host-side into [partition, free...]
    # SBUF layout so every input DMA is a maximal-linearity copy.
    keyT = nc.dram_tensor("keyT", [PART, ND, S], f16, kind="ExternalInput").ap()
    queryT = nc.dram_tensor("queryT", [PART, ND, T], f16, kind="ExternalInput").ap()
    wk = nc.dram_tensor("wk", [PART, ND, H], f16, kind="ExternalInput").ap()
    wq = nc.dram_tensor("wq", [PART, ND, H], f16, kind="ExternalInput").ap()
    bsum = nc.dram_tensor("bsum", [PART, NH], f32, kind="ExternalInput").ap()
    # vshift[p, h, j, m] = v[h*128+p] if j == m else 0
    bf16 = mybir.dt.bfloat16
    vshift = nc.dram_tensor("vshift", [PART, NH, 32, 32], bf16, kind="ExternalInput").ap()
    out = nc.dram_tensor("out", [T, S], f32, kind="ExternalOutput").ap()

    with tile.TileContext(nc) as tc:
        with (
            tc.tile_pool(name="const", bufs=1) as cpool,
            tc.tile_pool(name="epool", bufs=2) as epool,
            tc.tile_pool(name="zpool", bufs=2) as zpool,
            tc.tile_pool(name="spool", bufs=1) as spool,
            tc.tile_pool(name="kp_ps", bufs=1, space="PSUM") as kp_pool,
            tc.tile_pool(name="ps_small", bufs=2, space="PSUM") as ps_small,
        ):
            # ---- PE warmup: dep-free matmuls on uninitialized scratch keep
            # the PE busy during the input DMAs so HAM un-throttles to
            # 2.4 GHz before the real matmuls start.
            warm_w = cpool.tile([PART, 512], f16)
            nc.gpsimd.memset(warm_w[:], 0.0)
            warm_ps = kp_pool.tile([PART, 512], f32, tag="kp0", name="warm_ps")
            for _ in range(28):
                nc.tensor.matmul(
                    warm_ps[:], warm_w[:, 0:PART], warm_w[:],
                    start=True, stop=True, skip_group_check=True,
                )

            # ---- load inputs; qp path (scalar HWDGE queue) goes first ----
            queryT_sb = cpool.tile([PART, ND, T], f16)
            nc.scalar.dma_start(queryT_sb[:], queryT[:])
            wq_sb = cpool.tile([PART, ND, H], f16)
            nc.scalar.dma_start(wq_sb[:], wq[:])
            wk_sb = cpool.tile([PART, ND, H], f16)
            nc.sync.dma_start(wk_sb[:], wk[:])
            keyT_sb = cpool.tile([PART, ND, S], f16)
            for n in range(ND):
                eng = nc.sync if n < 2 else nc.scalar
                eng.dma_start(keyT_sb[:, n, :], keyT[:, n, :])
            bsum_sb = cpool.tile([PART, NH], f32)
            nc.gpsimd.dma_start(bsum_sb[:], bsum[:])
            vshift_sb = cpool.tile([PART, NH, 32, 32], bf16)
            nc.gpsimd.dma_start(vshift_sb[:], vshift[:])

            # ---- qp^T [h, t] first (small, unblocks the main loop) ----
            qp_sb = []
            for h in range(NH):
                qp_ps = ps_small.tile([PART, T], f32, tag="qp", name=f"qp_ps{h}")
                for n in range(ND):
                    nc.tensor.matmul(
                        qp_ps[:],
                        wq_sb[:, n, h * PART:(h + 1) * PART],
                        queryT_sb[:, n, :],
                        start=(n == 0),
                        stop=(n == ND - 1),
                    )
                q = cpool.tile([PART, T], f32, tag=f"qp_sb{h}", name=f"qp_sb{h}")
                # qp + (bk+bq), fused into the PSUM->SBUF copy
                nc.scalar.add(q[:], qp_ps[:], bsum_sb[:, h:h + 1])
                qp_sb.append(q)

            # ---- kp^T [h, s], PSUM-resident for the whole main loop ----
            kp_ps = [
                kp_pool.tile([PART, S], f32, tag=f"kp{h}", name=f"kp{h}")
                for h in range(NH)
            ]
            for h in range(NH):
                for c in range(NSC):
                    for n in range(ND):
                        nc.tensor.matmul(
                            kp_ps[h][:, c * 512:(c + 1) * 512],
                            wk_sb[:, n, h * PART:(h + 1) * PART],
                            keyT_sb[:, n, c * 512:(c + 1) * 512],
                            start=(n == 0),
                            stop=(n == ND - 1),
                        )

            # kp -> SBUF so the DVE z-adds run in 2x mode (PSUM src is 1x)
            kp_sb = []
            for h in range(NH):
                ksb = cpool.tile([PART, S], f32, tag=f"kp_sb{h}", name=f"kp_sb{h}")
                nc.vector.tensor_copy(ksb[:], kp_ps[h][:])
                kp_sb.append(ksb)

            # ---- main loop, batched: DVE z = kp + qp_t (per-partition
            # scalar add), one mega-ACTIVATE tanh per batch (amortizes the
            # per-instruction overhead), PE reduces via shifted-v.
            scores_ps = [
                ps_small.tile([32, S], f32, tag="qp", name=f"scores{g}")
                for g in range(2)
            ]
            pairs = [(t, h) for t in range(T) for h in range(NH)]
            batches = [2, 4] + [8] * 14 + [4, 4, 2]
            assert sum(batches) == len(pairs)
            idx = 0
            for bi, bs in enumerate(batches):
                zb = zpool.tile([PART, 8, S], f32, tag="z", name=f"z{bi}")
                for k in range(bs):
                    t, h = pairs[idx + k]
                    nc.vector.tensor_scalar_add(
                        zb[:, k, :], kp_sb[h][:], qp_sb[h][:, t:t + 1]
                    )
                eb = epool.tile([PART, 8, S], bf16, tag="e", name=f"e{bi}")
                nc.scalar.activation(eb[:, 0:bs, :], zb[:, 0:bs, :], AF.Tanh)
                for k in range(bs):
                    t, h = pairs[idx + k]
                    g, j = divmod(t, 32)
                    for c in range(NSC):
                        nc.tensor.matmul(
                            scores_ps[g][:, c * 512:(c + 1) * 512],
                            vshift_sb[:, h, j, :],
                            eb[:, k, c * 512:(c + 1) * 512],
                            start=(j == 0 and h == 0),
                            stop=(j == 31 and h == NH - 1),
                        )
                idx += bs

            # ---- softmax over s (free axis), per 32-query group; group 0
            # is complete halfway through the main loop and overlaps it ----
            for g in range(2):
                negmax = spool.tile([32, 1], f32, tag=f"nm{g}", name=f"negmax{g}")
                nc.vector.tensor_reduce(
                    negmax[:], scores_ps[g][:], axis=AX.X, op=ALU.max, negate=True,
                )
                p_sb = spool.tile([32, S], f32, tag=f"p{g}", name=f"p_sb{g}")
                ssum = spool.tile([32, 1], f32, tag=f"ss{g}", name=f"ssum{g}")
                nc.scalar.activation(
                    p_sb[:], scores_ps[g][:], AF.Exp, bias=negmax[:],
                    accum_out=ssum[:],
                )
                rinv = spool.tile([32, 1], f32, tag=f"ri{g}", name=f"rinv{g}")
                nc.vector.reciprocal(rinv[:], ssum[:])
                out_sb = spool.tile([32, S], f32, tag=f"ob{g}", name=f"out_sb{g}")
                nc.vector.tensor_scalar_mul(out_sb[:], p_sb[:], rinv[:])
                nc.sync.dma_start(out[32 * g:32 * (g + 1), :], out_sb[:])

    nc.compile()
    return nc


def _get_nc():
    if "nc" not in _CACHE:
        _CACHE["nc"] = _build_nc()
    return _CACHE["nc"]


def _part_layout(mat):
    """(X, F) with X = n*128+p  ->  contiguous (128, n, F) fp16."""
    x, f = mat.shape
    n = x // PART
    return np.ascontiguousarray(
        mat.reshape(n, PART, f).transpose(1, 0, 2).astype(np.float16))


def _in_maps(key, query, Wk, bk, Wq, bq, v):
    key = np.asarray(key, dtype=np.float32)
    query = np.asarray(query, dtype=np.float32)
    keyT = [_part_layout(key[:, b, :].T) for b in range(B)]     # (128, ND, S)
    queryT = [_part_layout(query[:, b, :].T) for b in range(B)]  # (128, ND, T)
    wk = _part_layout(np.asarray(Wk, dtype=np.float32))
    wq = _part_layout(np.asarray(Wq, dtype=np.float32))
    bsum0 = np.asarray(bk, dtype=np.float32) + np.asarray(bq, dtype=np.float32)
    bsum = np.ascontiguousarray(bsum0.reshape(NH, PART).T)       # (128, NH)
    vv = np.asarray(v, dtype=np.float32)
    import ml_dtypes
    vshift = np.zeros((PART, NH, 32, 32), dtype=ml_dtypes.bfloat16)
    for h in range(NH):
        for j in range(32):
            vshift[:, h, j, j] = vv[h * PART:(h + 1) * PART]
    return [
        {
            "keyT": keyT[b], "queryT": queryT[b],
            "wk": wk, "wq": wq, "bsum": bsum, "vshift": vshift,
        }
        for b in range(N_CORES)
    ]


def kernel(key, query, Wk, bk, Wq, bq, v):
    from concourse.bass_utils import run_bass_kernel_spmd

    nc = _get_nc()
    in_maps = _in_maps(key, query, Wk, bk, Wq, bq, v)
    res = run_bass_kernel_spmd(nc, in_maps, core_ids=list(range(N_CORES)))
    return np.stack([res.results[b]["out"] for b in range(N_CORES)])


def _ensure_ntff_hook():
    """Provide antenv.axon_hooks (absent in this image) so that
    run_bass_kernel_spmd(trace=True) can drive NTFF profiling via the
    libaxon_pjrt.so C ABI directly."""
    import sys
    import types
    import ctypes
    import contextlib

    try:
        from antenv.axon_hooks import get_axon_ntff_profile_hook  # noqa: F401
        return
    except ImportError:
        pass

    import antenv

    holder = {}
    mod = types.ModuleType("antenv.axon_hooks")
    mod.set_axon_ntff_profile_hook = lambda h: holder.__setitem__("h", h)
    mod.get_axon_ntff_profile_hook = lambda: holder.get("h")
    sys.modules["antenv.axon_hooks"] = mod
    antenv.axon_hooks = mod

    so_path = "/opt/axon/libaxon_pjrt.so"
    lib = ctypes.CDLL(so_path)
    if not hasattr(lib, "axon_start_nrt_profile"):
        return
    lib.axon_start_nrt_profile.argtypes = [
        ctypes.POINTER(ctypes.c_int64),
        ctypes.c_size_t,
    ]
    lib.axon_start_nrt_profile.restype = ctypes.c_int64
    lib.axon_stop_nrt_profile.argtypes = [ctypes.c_char_p]
    lib.axon_stop_nrt_profile.restype = ctypes.c_int64

    @contextlib.contextmanager
    def _hook(output_dir, device_ids):
        import jax

        jax.devices()
        if device_ids:
            ids = (ctypes.c_int64 * len(device_ids))(*device_ids)
            rc = lib.axon_start_nrt_profile(ids, len(device_ids))
        else:
            rc = lib.axon_start_nrt_profile(None, 0)
        if rc != 0:
            raise RuntimeError(f"axon_start_nrt_profile rc={rc}")
        try:
            yield
        finally:
            n = lib.axon_stop_nrt_profile(str(output_dir).encode())
            print(f"ntff profile: {n} file(s) written to {output_dir}")

    mod.set_axon_ntff_profile_hook(_hook)


def kernel_traced(key, query, Wk, bk, Wq, bq, v):
    """Same as kernel() but captures the neuron profile; returns
    (output, exec_time_ns, trace_path)."""
    from concourse.bass_utils import run_bass_kernel_spmd

    _ensure_ntff_hook()
    nc = _get_nc()
    in_maps = _in_maps(key, query, Wk, bk, Wq, bq, v)
    res = run_bass_kernel_spmd(
        nc, in_maps, core_ids=list(range(N_CORES)), trace=True
    )
    outp = np.stack([res.results[b]["out"] for b in range(N_CORES)])
    trace_path = None
    if res.instructions_and_trace is not None:
        trace_path = res.instructions_and_trace[1]
    return outp, res.exec_time_ns, trace_path


# revision 20
# speedup vs baseline: 1.1952x; 1.0130x over previous
"""Additive (Bahdanau) attention on 8 Trainium2 NeuronCores.

Reference computation (per batch b):
    kp = key[:, b, :] @ Wk            (S, H)
    qp = query[:, b, :] @ Wq + bk+bq  (T, H)
    scores[t, s] = sum_h v[h] * tanh(qp[t, h] + kp[s, h])
    out[b] = softmax(scores, axis=s)  (T, S)

Sharding: batch B=8 across the 8 cores, one batch element per core.
No collectives needed.

Device layout: H on SBUF partitions (2 halves of 128).  The broadcast
add + tanh is fused into a single ScalarE ACTIVATE per (t, half) using
the per-partition bias operand: E = tanh(kp[h, s] + qp_t[h]), reading
kp straight from PSUM.  The reduction over h runs on the TensorEngine
with a shifted-v stationary operand (v in column t%32 routes score_t to
PSUM partition t); softmax is fused via reduce_max(negate) +
Exp(accum_out).  PE operands are fp16 (measured end-to-end rel err
1.5e-3 on the reference data): fp32 matmuls lower to two HW passes,
fp16 to one.
"""

import numpy as np

S, T, B = 1024, 64, 8
D, H = 512, 256
N_CORES = 8
PART = 128
ND = D // PART       # 4 contraction chunks
NH = H // PART       # 2 h halves
NSC = S // 512       # matmul N<=512 chunks (PSUM bank limit)

_CACHE: dict = {}


def _build_nc():
    import concourse.bass as bass
    import concourse.tile as tile
    from concourse import bacc, mybir

    f32 = mybir.dt.float32
    f16 = mybir.dt.float16
    AF = mybir.ActivationFunctionType
    AX = mybir.AxisListType
    ALU = mybir.AluOpType

    nc = bacc.Bacc(
        "TRN2",
        target_bir_lowering=False,
        debug=False,
        num_devices=N_CORES,
    )

    # All inputs arrive pre-arranged host-side into [partition, free...]
    # SBUF layout so every input DMA is a maximal-linearity copy.
    keyT = nc.dram_tensor("keyT", [PART, ND, S], f16, kind="ExternalInput").ap()
    queryT = nc.dram_tensor("queryT", [PART, ND, T], f16, kind="ExternalInput").ap()
    wk = nc.dram_tensor("wk", [PART, ND, H], f16, kind="ExternalInput").ap()
    wq = nc.dram_tensor("wq", [PART, ND, H], f16, kind="ExternalInput").ap()
    bsum = nc.dram_tensor("bsum", [PART, NH], f32, kind="ExternalInput").ap()
    # vshift[p, h, j, m] = v[h*128+p] if j == m else 0
    bf16 = mybir.dt.bfloat16
    vshift = nc.dram_tensor("vshift", [PART, NH, 32, 32], bf16, kind="ExternalInput").ap()
    out = nc.dram_tensor("out", [T, S], f32, kind="ExternalOutput").ap()

    with tile.TileContext(nc) as tc:
        with (
            tc.tile_pool(name="const", bufs=1) as cpool,
            tc.tile_pool(name="epool", bufs=2) as epool,
            tc.tile_pool(name="zpool", bufs=2) as zpool,
            tc.tile_pool(name="spool", bufs=1) as spool,
            tc.tile_pool(name="kp_ps", bufs=1, space="PSUM") as kp_pool,
            tc.tile_pool(name="ps_small", bufs=2, space="PSUM") as ps_small,
        ):
            # ---- PE warmup: dep-free matmuls on uninitialized scratch keep
            # the PE busy during the input DMAs so HAM un-throttles to
            # 2.4 GHz before the real matmuls start.
            warm_w = cpool.tile([PART, 512], f16)
            nc.gpsimd.memset(warm_w[:], 0.0)
            warm_ps = kp_pool.tile([PART, 512], f32, tag="kp0", name="warm_ps")
            for _ in range(28):
                nc.tensor.matmul(
                    warm_ps[:], warm_w[:, 0:PART], warm_w[:],
                    start=True, stop=True, skip_group_check=True,
                )

            # ---- load inputs; qp path (scalar HWDGE queue) goes first ----
            queryT_sb = cpool.tile([PART, ND, T], f16)
            nc.scalar.dma_start(queryT_sb[:], queryT[:])
            wq_sb = cpool.tile([PART, ND, H], f16)
            nc.scalar.dma_start(wq_sb[:], wq[:])
            wk_sb = cpool.tile([PART, ND, H], f16)
            nc.sync.dma_start(wk_sb[:], wk[:])
            keyT_sb = cpool.tile([PART, ND, S], f16)
            for n, eng in zip(range(ND), (nc.sync, nc.sync, nc.gpsimd, nc.gpsimd)):
                eng.dma_start(keyT_sb[:, n, :], keyT[:, n, :])
            bsum_sb = cpool.tile([PART, NH], f32)
            nc.gpsimd.dma_start(bsum_sb[:], bsum[:])
            vshift_sb = cpool.tile([PART, NH, 32, 32], bf16)
            nc.gpsimd.dma_start(vshift_sb[:], vshift[:])

            # ---- qp^T [h, t] first (small, unblocks the main loop) ----
            qp_sb = []
            for h in range(NH):
                qp_ps = ps_small.tile([PART, T], f32, tag="qp", name=f"qp_ps{h}")
                for n in range(ND):
                    nc.tensor.matmul(
                        qp_ps[:],
                        wq_sb[:, n, h * PART:(h + 1) * PART],
                        queryT_sb[:, n, :],
                        start=(n == 0),
                        stop=(n == ND - 1),
                    )
                q = cpool.tile([PART, T], f32, tag=f"qp_sb{h}", name=f"qp_sb{h}")
                # qp + (bk+bq), fused into the PSUM->SBUF copy
                nc.scalar.add(q[:], qp_ps[:], bsum_sb[:, h:h + 1])
                qp_sb.append(q)

            # ---- kp^T [h, s], PSUM-resident for the whole main loop ----
            kp_ps = [
                kp_pool.tile([PART, S], f32, tag=f"kp{h}", name=f"kp{h}")
                for h in range(NH)
            ]
            for h in range(NH):
                for c in range(NSC):
                    for n in range(ND):
                        nc.tensor.matmul(
                            kp_ps[h][:, c * 512:(c + 1) * 512],
                            wk_sb[:, n, h * PART:(h + 1) * PART],
                            keyT_sb[:, n, c * 512:(c + 1) * 512],
                            start=(n == 0),
                            stop=(n == ND - 1),
                        )

            # kp -> SBUF so the DVE z-adds run in 2x mode (PSUM src is 1x)
            kp_sb = []
            for h in range(NH):
                ksb = cpool.tile([PART, S], f16, tag=f"kp_sb{h}", name=f"kp_sb{h}")
                nc.vector.tensor_copy(ksb[:], kp_ps[h][:])
                kp_sb.append(ksb)

            # ---- main loop, batched: DVE z = kp + qp_t (per-partition
            # scalar add), one mega-ACTIVATE tanh per batch (amortizes the
            # per-instruction overhead), PE reduces via shifted-v.
            scores_ps = [
                ps_small.tile([32, S], f32, tag="qp", name=f"scores{g}")
                for g in range(2)
            ]
            pairs = [(t, h) for t in range(T) for h in range(NH)]
            batches = [2, 4] + [8] * 14 + [4, 4, 2]
            assert sum(batches) == len(pairs)
            idx = 0
            for bi, bs in enumerate(batches):
                zb = zpool.tile([PART, 8, S], f16, tag="z", name=f"z{bi}")
                for k in range(bs):
                    t, h = pairs[idx + k]
                    nc.vector.tensor_scalar_add(
                        zb[:, k, :], kp_sb[h][:], qp_sb[h][:, t:t + 1]
                    )
                eb = epool.tile([PART, 8, S], bf16, tag="e", name=f"e{bi}")
                nc.scalar.activation(eb[:, 0:bs, :], zb[:, 0:bs, :], AF.Tanh)
                for k in range(bs):
                    t, h = pairs[idx + k]
                    g, j = divmod(t, 32)
                    for c in range(NSC):
                        nc.tensor.matmul(
                            scores_ps[g][:, c * 512:(c + 1) * 512],
                            vshift_sb[:, h, j, :],
                            eb[:, k, c * 512:(c + 1) * 512],
                            start=(j == 0 and h == 0),
                            stop=(j == 31 and h == NH - 1),
                        )
                idx += bs

            # ---- softmax over s (free axis), per 32-query group; group 0
            # is complete halfway through the main loop and overlaps it ----
            for g in range(2):
                negmax = spool.tile([32, 1], f32, tag=f"nm{g}", name=f"negmax{g}")
                nc.vector.tensor_reduce(
                    negmax[:], scores_ps[g][:], axis=AX.X, op=ALU.max, negate=True,
                )
                p_sb = spool.tile([32, S], f32, tag=f"p{g}", name=f"p_sb{g}")
                ssum = spool.tile([32, 1], f32, tag=f"ss{g}", name=f"ssum{g}")
                nc.scalar.activation(
                    p_sb[:], scores_ps[g][:], AF.Exp, bias=negmax[:],
                    accum_out=ssum[:],
                )
                rinv = spool.tile([32, 1], f32, tag=f"ri{g}", name=f"rinv{g}")
                nc.vector.reciprocal(rinv[:], ssum[:])
                out_sb = spool.tile([32, S], f32, tag=f"ob{g}", name=f"out_sb{g}")
                nc.vector.tensor_scalar_mul(out_sb[:], p_sb[:], rinv[:])
                nc.sync.dma_start(out[32 * g:32 * (g + 1), :], out_sb[:])

    nc.compile()
    return nc


def _get_nc():
    if "nc" not in _CACHE:
        _CACHE["nc"] = _build_nc()
    return _CACHE["nc"]


def _part_layout(mat):
    """(X, F) with X = n*128+p  ->  contiguous (128, n, F) fp16."""
    x, f = mat.shape
    n = x // PART
    return np.ascontiguousarray(
        mat.reshape(n, PART, f).transpose(1, 0, 2).astype(np.float16))


def _in_maps(key, query, Wk, bk, Wq, bq, v):
    key = np.asarray(key, dtype=np.float32)
    query = np.asarray(query, dtype=np.float32)
    keyT = [_part_layout(key[:, b, :].T) for b in range(B)]     # (128, ND, S)
    queryT = [_part_layout(query[:, b, :].T) for b in range(B)]  # (128, ND, T)
    wk = _part_layout(np.asarray(Wk, dtype=np.float32))
    wq = _part_layout(np.asarray(Wq, dtype=np.float32))
    bsum0 = np.asarray(bk, dtype=np.float32) + np.asarray(bq, dtype=np.float32)
    bsum = np.ascontiguousarray(bsum0.reshape(NH, PART).T)       # (128, NH)
    vv = np.asarray(v, dtype=np.float32)
    import ml_dtypes
    vshift = np.zeros((PART, NH, 32, 32), dtype=ml_dtypes.bfloat16)
    for h in range(NH):
        for j in range(32):
            vshift[:, h, j, j] = vv[h * PART:(h + 1) * PART]
    return [
        {
            "keyT": keyT[b], "queryT": queryT[b],
            "wk": wk, "wq": wq, "bsum": bsum, "vshift": vshift,
        }
        for b in range(N_CORES)
    ]


def kernel(key, query, Wk, bk, Wq, bq, v):
    from concourse.bass_utils import run_bass_kernel_spmd

    nc = _get_nc()
    in_maps = _in_maps(key, query, Wk, bk, Wq, bq, v)
    res = run_bass_kernel_spmd(nc, in_maps, core_ids=list(range(N_CORES)))
    return np.stack([res.results[b]["out"] for b in range(N_CORES)])


def _ensure_ntff_hook():
    """Provide antenv.axon_hooks (absent in this image) so that
    run_bass_kernel_spmd(trace=True) can drive NTFF profiling via the
    libaxon_pjrt.so C ABI directly."""
    import sys
    import types
    import ctypes
    import contextlib

    try:
        from antenv.axon_hooks import get_axon_ntff_profile_hook  # noqa: F401
        return
    except ImportError:
        pass

    import antenv

    holder = {}
    mod = types.ModuleType("antenv.axon_hooks")
    mod.set_axon_ntff_profile_hook = lambda h: holder.__setitem__("h", h)
    mod.get_axon_ntff_profile_hook = lambda: holder.get("h")
    sys.modules["antenv.axon_hooks"] = mod
    antenv.axon_hooks = mod

    so_path = "/opt/axon/libaxon_pjrt.so"
    lib = ctypes.CDLL(so_path)
    if not hasattr(lib, "axon_start_nrt_profile"):
        return
    lib.axon_start_nrt_profile.argtypes = [
        ctypes.POINTER(ctypes.c_int64),
        ctypes.c_size_t,
    ]
    lib.axon_start_nrt_profile.restype = ctypes.c_int64
    lib.axon_stop_nrt_profile.argtypes = [ctypes.c_char_p]
    lib.axon_stop_nrt_profile.restype = ctypes.c_int64

    @contextlib.contextmanager
    def _hook(output_dir, device_ids):
        import jax

        jax.devices()
        if device_ids:
            ids = (ctypes.c_int64 * len(device_ids))(*device_ids)
            rc = lib.axon_start_nrt_profile(ids, len(device_ids))
        else:
            rc = lib.axon_start_nrt_profile(None, 0)
        if rc != 0:
            raise RuntimeError(f"axon_start_nrt_profile rc={rc}")
        try:
            yield
        finally:
            n = lib.axon_stop_nrt_profile(str(output_dir).encode())
            print(f"ntff profile: {n} file(s) written to {output_dir}")

    mod.set_axon_ntff_profile_hook(_hook)


def kernel_traced(key, query, Wk, bk, Wq, bq, v):
    """Same as kernel() but captures the neuron profile; returns
    (output, exec_time_ns, trace_path)."""
    from concourse.bass_utils import run_bass_kernel_spmd

    _ensure_ntff_hook()
    nc = _get_nc()
    in_maps = _in_maps(key, query, Wk, bk, Wq, bq, v)
    res = run_bass_kernel_spmd(
        nc, in_maps, core_ids=list(range(N_CORES)), trace=True
    )
    outp = np.stack([res.results[b]["out"] for b in range(N_CORES)])
    trace_path = None
    if res.instructions_and_trace is not None:
        trace_path = res.instructions_and_trace[1]
    return outp, res.exec_time_ns, trace_path
